# revision 2
# baseline (speedup 1.0000x reference)
"""AtomicOrbitals kernel for Trainium2 (8 NeuronCores, data-parallel over batch).

Math: for electron position p and basis j (atom a_j, exponent alpha_j,
angular momentum l_j/m_j, radial power n_j, weight K_j = norm_cst*coeffs):

    bas_j(p) = K_j * Y~_j(p - c_{a_j}) * r^{g_j} * exp(-alpha_j r^2)
    ao[:, index_ctr[j]] += bas_j

where Y~ is the angular polynomial (degree <= 2) WITHOUT the 1/r^l_eff
factor and g_j = n_j - l_eff_j (zero for standard GTOs).

v5 device decomposition (per core: 8192 electrons, 256 bases in two
128-basis halves h). PE row-tiling: the exponent-argument matmul runs as
a K=96 tile on array rows 0-95 while the angular-polynomial matmul runs
as a K=32 tile on rows 96-127 -- concurrently (tile_position row groups),
with rhs features living on the matching SBUF partitions:

  rows  0..95 : r2 per atom (hi/lo/hi bf16 levels x alpha hi/hi/lo), plus
                ln(r2) levels when bas_n != l (r^g factor)
  rows 96..127: monomials phi (hi/lo/hi bf16 levels)

Per 512-electron chunk c (elec block [e, e+512)):
  T[:,0:512] = WT_h0^T @ rhs_t      T[:,512:1024] = WT_h1^T @ rhs_t
  P[:,0:512] = WP_h0^T @ rhs_p      P[:,512:1024] = WP_h1^T @ rhs_p
  u   = exp(T)        (ScalarE, one [128,1024] instr, f16 out)
  bas = P * u         (VectorE, one [128,1024] instr, f16 out)
  DMA bas -> out[:, c*1024:(c+1)*1024]

PE wall time is ~0.7us/16 chunks thanks to tile concurrency, so ScalarE's
exp (1 elem/cycle/lane @1.2GHz, (N+352) cycles/instr) sets the cadence;
the DVE mul (1x mode, PSUM operand) runs just under it.

The 0/1-scatter onto 128 orbitals stays on the HOST: index_ctr is sorted,
so ao = segment-sum over contiguous basis groups (np.add.reduceat).

Startup: a dummy exp is issued first so the ~2.7us ACT Exp-table load
overlaps the input DMA; no PE warm-up needed (PE has 2x slack even at the
1.2GHz low-K clock). Host zero-fills unused feature rows, so no on-device
pad memsets.
"""

import sys
import numpy as np

sys.path.insert(0, "/opt/trn_rl_repo")

NBATCH, NELEC, NATOMS, NBAS, NORB = 1024, 64, 16, 256, 128
N_CORES = 8
BPC = NBATCH // N_CORES          # batch rows per core
EPC = BPC * NELEC                # electrons per core (8192)
EBLK = 512                       # electrons per chunk
NCHUNK = EPC // EBLK             # 16
NTOT = NBATCH * NELEC

KT = 96                          # t-feature rows (partitions 0..95)
KP = 32                          # phi rows (partitions 96..127)

C0 = 0.2820948
C1 = 0.4886025
C2XY = 1.0925484
C2Z2 = 0.31539156
C2D = 0.5462742

_compiled = {}


def _split_hilo(x, bf16):
    hi = x.astype(bf16)
    lo = (x - hi.astype(np.float64)).astype(bf16)
    return hi, lo


def _host_build(input, atom_coords, bas_exp, bas_coeffs, norm_cst, bas_n,
                bas_l, bas_m, bas_atom_index, index_ctr):
    import ml_dtypes
    bf16 = ml_dtypes.bfloat16

    p = np.asarray(input, np.float64).reshape(NTOT, 3)
    x, y, z = p[:, 0], p[:, 1], p[:, 2]
    ac = np.asarray(atom_coords, np.float64)
    alpha = np.asarray(bas_exp, np.float64)
    K = np.asarray(norm_cst, np.float64) * np.asarray(bas_coeffs, np.float64)
    n_j = np.asarray(bas_n, np.float64)
    l_j = np.asarray(bas_l, np.int64)
    m_j = np.asarray(bas_m, np.int64)
    a_j = np.asarray(bas_atom_index, np.int64)

    # monomial features [10, NTOT]: 1, x, y, z, x2, y2, z2, xy, xz, yz
    phi = np.stack([np.ones_like(x), x, y, z, x * x, y * y, z * z,
                    x * y, x * z, y * z])

    # per-atom squared distances [NATOMS, NTOT]
    d = p[None, :, :] - ac[:, None, :]
    r2A = np.einsum("anc,anc->an", d, d)

    # per-basis angular polynomial in absolute monomials, times K_j
    W = np.zeros((10, NBAS))
    cx, cy, cz = ac[a_j, 0], ac[a_j, 1], ac[a_j, 2]
    l_eff = np.where(l_j == 0, 0, np.where(l_j == 1, 1, 2))
    for j in range(NBAS):
        w = np.zeros(10)
        bx, by, bz = cx[j], cy[j], cz[j]
        if l_eff[j] == 0:
            w[0] = C0
        elif l_eff[j] == 1:
            if m_j[j] == -1:
                w[2], w[0] = C1, -C1 * by
            elif m_j[j] == 0:
                w[3], w[0] = C1, -C1 * bz
            else:
                w[1], w[0] = C1, -C1 * bx
        else:
            m = m_j[j]
            if m == -2:      # C2XY * xc * yc
                w[7] = C2XY
                w[1] = -C2XY * by
                w[2] = -C2XY * bx
                w[0] = C2XY * bx * by
            elif m == -1:    # C2XY * yc * zc
                w[9] = C2XY
                w[2] = -C2XY * bz
                w[3] = -C2XY * by
                w[0] = C2XY * by * bz
            elif m == 0:     # C2Z2 * (2 zc^2 - xc^2 - yc^2)
                w[6], w[4], w[5] = 2 * C2Z2, -C2Z2, -C2Z2
                w[3], w[1], w[2] = -4 * C2Z2 * bz, 2 * C2Z2 * bx, 2 * C2Z2 * by
                w[0] = C2Z2 * (2 * bz * bz - bx * bx - by * by)
            elif m == 1:     # C2XY * zc * xc
                w[8] = C2XY
                w[1] = -C2XY * bz
                w[3] = -C2XY * bx
                w[0] = C2XY * bx * bz
            else:            # C2D * (xc^2 - yc^2)
                w[4], w[5] = C2D, -C2D
                w[1], w[2] = -2 * C2D * bx, 2 * C2D * by
                w[0] = C2D * (bx * bx - by * by)
        W[:, j] = K[j] * w

    g = n_j - l_eff
    lean = bool(np.all(np.abs(g) < 1e-12))

    phi_h, phi_l = _split_hilo(phi, bf16)
    r2_h, r2_l = _split_hilo(r2A, bf16)

    onehot = np.zeros((NATOMS, NBAS))
    onehot[a_j, np.arange(NBAS)] = 1.0

    W_h = W.astype(bf16)
    W_l = (W - W_h.astype(np.float64)).astype(bf16)

    ah = alpha.astype(bf16)
    al = (alpha - ah.astype(np.float64)).astype(np.float64)

    # t block (rows 0..KT): -(ah+al)(r2h+r2l) to ~2^-16 relative, plus
    # q*ln(r2) levels for the r^g factor when not lean
    rows_t = [r2_h, r2_l, r2_h]                           # 48 rows
    wt_blocks = [onehot * (-ah.astype(np.float64)),
                 onehot * (-ah.astype(np.float64)),
                 onehot * (-al)]
    if not lean:
        lnA = np.log(np.maximum(r2A, 1e-300))
        ln_h, ln_l = _split_hilo(lnA, bf16)
        q = 0.5 * g
        qh = q.astype(bf16)
        ql = (q - qh.astype(np.float64)).astype(np.float64)
        rows_t += [ln_h, ln_l]                            # +32 rows
        wt_blocks += [onehot * qh.astype(np.float64),
                      onehot * qh.astype(np.float64)]
        if not np.allclose(ql, 0):
            # fold the residual-q * ln_h correction into the last level
            wt_blocks[-1] = onehot * (qh.astype(np.float64) + ql)

    rows_p = [phi_h, phi_l, phi_h]                        # 30 rows
    wp_blocks = [W_h, W_h, W_l]

    WT = np.concatenate(wt_blocks).astype(bf16)           # [<=80, NBAS]
    WP = np.concatenate(wp_blocks).astype(bf16)           # [30, NBAS]
    nt, npr = WT.shape[0], WP.shape[0]
    assert nt <= KT and npr <= KP

    rhs = np.zeros((128, NTOT), bf16)
    rhs[0:nt] = np.concatenate(rows_t)
    rhs[KT:KT + npr] = np.concatenate(rows_p)

    # weights: [128, 256] -- cols h*128..h*128+128 hold half h; partitions
    # 0..KT-1 are WT rows, KT.. are WP rows (matching rhs partitions)
    WB = np.zeros((128, 2 * 128), bf16)
    for h in range(2):
        WB[0:nt, h * 128:(h + 1) * 128] = WT[:, h * 128:(h + 1) * 128]
        WB[KT:KT + npr, h * 128:(h + 1) * 128] = WP[:, h * 128:(h + 1) * 128]

    return np.ascontiguousarray(rhs), np.ascontiguousarray(WB)


def _build_nc():
    import concourse.bacc as bacc
    import concourse.mybir as mybir
    import concourse.tile as tile

    f32 = mybir.dt.float32
    f16 = mybir.dt.float16
    bf = mybir.dt.bfloat16

    nc = bacc.Bacc("TRN2", target_bir_lowering=False, debug=False,
                   num_devices=N_CORES)
    rhs_d = nc.dram_tensor("rhs", [128, EPC], bf, kind="ExternalInput")
    wb_d = nc.dram_tensor("wb", [128, 256], bf, kind="ExternalInput")
    out_d = nc.dram_tensor("out", [128, 2 * EPC], f16, kind="ExternalOutput")

    with tile.TileContext(nc) as tc:
        with (
            tc.tile_pool(name="wpool", bufs=1) as wpool,
            tc.tile_pool(name="inpool", bufs=1) as inpool,
            tc.tile_pool(name="upool", bufs=4) as upool,
            tc.tile_pool(name="baspool", bufs=8) as baspool,
            tc.tile_pool(name="ps", bufs=4, space="PSUM") as ps,
        ):
            rt = inpool.tile([128, EPC], bf, tag="rt")
            wb_t = wpool.tile([128, 256], bf, tag="wb")
            warm = wpool.tile([128, 32], bf, tag="warm")
            udum = wpool.tile([128, 32], f32, tag="udum")

            # ACT Exp table preload (~2.7us) overlaps the input DMA wait:
            # memset a tiny tile on gpsimd, exp it on scalar immediately
            nc.gpsimd.memset(warm[:], 0.0)
            nc.scalar.activation(udum[:], warm[:],
                                 mybir.ActivationFunctionType.Exp)

            # startup loads: weights first (tiny), then rhs in rising sizes
            nc.sync.dma_start(wb_t[:], wb_d[:])
            nc.scalar.dma_start(rt[:, 0:EBLK], rhs_d[:, 0:EBLK])
            nc.scalar.dma_start(rt[:, EBLK:2 * EBLK],
                                rhs_d[:, EBLK:2 * EBLK])
            nc.sync.dma_start(rt[:, 2 * EBLK:4 * EBLK],
                              rhs_d[:, 2 * EBLK:4 * EBLK])
            nc.scalar.dma_start(rt[:, 4 * EBLK:8 * EBLK],
                                rhs_d[:, 4 * EBLK:8 * EBLK])
            nc.sync.dma_start(rt[:, 8 * EBLK:], rhs_d[:, 8 * EBLK:])

            for c in range(NCHUNK):
                es = c * EBLK
                tt = ps.tile([128, 2 * EBLK], f32, tag="ps")
                for h in range(2):
                    # K=96 tile on PE rows 0..95 (t): runs concurrently
                    # with the K=32 tile below (rows 96..127)
                    nc.tensor.matmul(tt[:, h * EBLK:(h + 1) * EBLK],
                                     wb_t[0:KT, h * 128:(h + 1) * 128],
                                     rt[0:KT, es:es + EBLK],
                                     start=True, stop=True,
                                     tile_position=(0, 0))
                pt = ps.tile([128, 2 * EBLK], f32, tag="ps")
                for h in range(2):
                    nc.tensor.matmul(pt[:, h * EBLK:(h + 1) * EBLK],
                                     wb_t[KT:128, h * 128:(h + 1) * 128],
                                     rt[KT:128, es:es + EBLK],
                                     start=True, stop=True,
                                     tile_position=(KT, 0))
                u = upool.tile([128, 2 * EBLK], f16, tag="u")
                nc.scalar.activation(u[:], tt[:],
                                     mybir.ActivationFunctionType.Exp)
                b = baspool.tile([128, 2 * EBLK], f16, tag="bas")
                nc.vector.tensor_mul(b[:], pt[:], u[:])
                nc.sync.dma_start(out_d[:, 2 * es:2 * es + 2 * EBLK], b[:])

    nc.compile()
    return nc


def kernel(input, atom_coords, bas_exp, bas_coeffs, norm_cst, bas_n,
           bas_l, bas_m, bas_atom_index, index_ctr, _res_hook=None):
    from concourse.bass_utils import run_bass_kernel_spmd

    rhs, WB = _host_build(
        input, atom_coords, bas_exp, bas_coeffs, norm_cst, bas_n,
        bas_l, bas_m, bas_atom_index, index_ctr)

    if "nc" not in _compiled:
        _compiled["nc"] = _build_nc()
    nc = _compiled["nc"]

    in_maps = []
    for i in range(N_CORES):
        es = slice(i * EPC, (i + 1) * EPC)
        in_maps.append({
            "rhs": np.ascontiguousarray(rhs[:, es]),
            "wb": WB,
        })

    res = run_bass_kernel_spmd(nc, in_maps, list(range(N_CORES)))
    if _res_hook is not None:
        _res_hook(res)

    # host-side scatter: index_ctr is sorted, so each orbital's bases are
    # a contiguous run -> segment sums via reduceat
    ictr = np.asarray(index_ctr, np.int64)
    present, first = np.unique(ictr, return_index=True)

    out = np.empty((NBATCH, NELEC, NORB), np.float32)
    for i in range(N_CORES):
        blk = res.results[i]["out"]                  # [128, 2*EPC] f16
        # chunk layout: [128, NCHUNK, 2 halves, EBLK] -> [256, EPC]
        bas = np.ascontiguousarray(
            blk.reshape(128, NCHUNK, 2, EBLK).transpose(2, 0, 1, 3)
        ).reshape(NBAS, EPC).astype(np.float32)
        sums = np.add.reduceat(bas, first, axis=0)   # [npresent, EPC]
        ao = np.zeros((NORB, EPC), np.float32)
        ao[present] = sums
        out[i * BPC:(i + 1) * BPC] = ao.T.reshape(BPC, NELEC, NORB)
    return out


# revision 6
# speedup vs baseline: 1.2945x; 1.2945x over previous
"""AtomicOrbitals kernel for Trainium2 (8 NeuronCores, data-parallel over batch).

Math: for electron position p and basis j (atom a_j, exponent alpha_j,
angular momentum l_j/m_j, radial power n_j, weight K_j = norm_cst*coeffs):

    bas_j(p) = K_j * Y~_j(p - c_{a_j}) * r^{g_j} * exp(-alpha_j r^2)
    ao[:, index_ctr[j]] += bas_j

where Y~ is the angular polynomial (degree <= 2) WITHOUT the 1/r^l_eff
factor and g_j = n_j - l_eff_j (zero for standard GTOs).

v5 device decomposition (per core: 8192 electrons, 256 bases in two
128-basis halves h). PE row-tiling: the exponent-argument matmul runs as
a K=96 tile on array rows 0-95 while the angular-polynomial matmul runs
as a K=32 tile on rows 96-127 -- concurrently (tile_position row groups),
with rhs features living on the matching SBUF partitions:

  rows  0..95 : r2 per atom (hi/lo/hi bf16 levels x alpha hi/hi/lo), plus
                ln(r2) levels when bas_n != l (r^g factor)
  rows 96..127: monomials phi (hi/lo/hi bf16 levels)

Per 512-electron chunk c (elec block [e, e+512)):
  T[:,0:512] = WT_h0^T @ rhs_t      T[:,512:1024] = WT_h1^T @ rhs_t
  P[:,0:512] = WP_h0^T @ rhs_p      P[:,512:1024] = WP_h1^T @ rhs_p
  u   = exp(T)        (ScalarE, one [128,1024] instr, f16 out)
  bas = P * u         (VectorE, one [128,1024] instr, f16 out)
  DMA bas -> out[:, c*1024:(c+1)*1024]

PE wall time is ~0.7us/16 chunks thanks to tile concurrency, so ScalarE's
exp (1 elem/cycle/lane @1.2GHz, (N+352) cycles/instr) sets the cadence;
the DVE mul (1x mode, PSUM operand) runs just under it.

The 0/1-scatter onto 128 orbitals stays on the HOST: index_ctr is sorted,
so ao = segment-sum over contiguous basis groups (np.add.reduceat).

Startup: a dummy exp is issued first so the ~2.7us ACT Exp-table load
overlaps the input DMA; no PE warm-up needed (PE has 2x slack even at the
1.2GHz low-K clock). Host zero-fills unused feature rows, so no on-device
pad memsets.
"""

import sys
import numpy as np

sys.path.insert(0, "/opt/trn_rl_repo")

NBATCH, NELEC, NATOMS, NBAS, NORB = 1024, 64, 16, 256, 128
N_CORES = 8
BPC = NBATCH // N_CORES          # batch rows per core
EPC = BPC * NELEC                # electrons per core (8192)
EBLK = 512                       # electrons per chunk
NCHUNK = EPC // EBLK             # 16
NTOT = NBATCH * NELEC

KT = 96                          # t-feature rows (partitions 0..95)
KP = 32                          # phi rows (partitions 96..127)

C0 = 0.2820948
C1 = 0.4886025
C2XY = 1.0925484
C2Z2 = 0.31539156
C2D = 0.5462742

_compiled = {}


def _split_hilo(x, bf16):
    hi = x.astype(bf16)
    lo = (x - hi.astype(np.float64)).astype(bf16)
    return hi, lo


def _host_build(input, atom_coords, bas_exp, bas_coeffs, norm_cst, bas_n,
                bas_l, bas_m, bas_atom_index, index_ctr):
    import ml_dtypes
    bf16 = ml_dtypes.bfloat16

    p = np.asarray(input, np.float64).reshape(NTOT, 3)
    x, y, z = p[:, 0], p[:, 1], p[:, 2]
    ac = np.asarray(atom_coords, np.float64)
    alpha = np.asarray(bas_exp, np.float64)
    K = np.asarray(norm_cst, np.float64) * np.asarray(bas_coeffs, np.float64)
    n_j = np.asarray(bas_n, np.float64)
    l_j = np.asarray(bas_l, np.int64)
    m_j = np.asarray(bas_m, np.int64)
    a_j = np.asarray(bas_atom_index, np.int64)

    # monomial features [10, NTOT]: 1, x, y, z, x2, y2, z2, xy, xz, yz
    phi = np.stack([np.ones_like(x), x, y, z, x * x, y * y, z * z,
                    x * y, x * z, y * z])

    # per-atom squared distances [NATOMS, NTOT]
    d = p[None, :, :] - ac[:, None, :]
    r2A = np.einsum("anc,anc->an", d, d)

    # per-basis angular polynomial in absolute monomials, times K_j
    W = np.zeros((10, NBAS))
    cx, cy, cz = ac[a_j, 0], ac[a_j, 1], ac[a_j, 2]
    l_eff = np.where(l_j == 0, 0, np.where(l_j == 1, 1, 2))
    for j in range(NBAS):
        w = np.zeros(10)
        bx, by, bz = cx[j], cy[j], cz[j]
        if l_eff[j] == 0:
            w[0] = C0
        elif l_eff[j] == 1:
            if m_j[j] == -1:
                w[2], w[0] = C1, -C1 * by
            elif m_j[j] == 0:
                w[3], w[0] = C1, -C1 * bz
            else:
                w[1], w[0] = C1, -C1 * bx
        else:
            m = m_j[j]
            if m == -2:      # C2XY * xc * yc
                w[7] = C2XY
                w[1] = -C2XY * by
                w[2] = -C2XY * bx
                w[0] = C2XY * bx * by
            elif m == -1:    # C2XY * yc * zc
                w[9] = C2XY
                w[2] = -C2XY * bz
                w[3] = -C2XY * by
                w[0] = C2XY * by * bz
            elif m == 0:     # C2Z2 * (2 zc^2 - xc^2 - yc^2)
                w[6], w[4], w[5] = 2 * C2Z2, -C2Z2, -C2Z2
                w[3], w[1], w[2] = -4 * C2Z2 * bz, 2 * C2Z2 * bx, 2 * C2Z2 * by
                w[0] = C2Z2 * (2 * bz * bz - bx * bx - by * by)
            elif m == 1:     # C2XY * zc * xc
                w[8] = C2XY
                w[1] = -C2XY * bz
                w[3] = -C2XY * bx
                w[0] = C2XY * bx * bz
            else:            # C2D * (xc^2 - yc^2)
                w[4], w[5] = C2D, -C2D
                w[1], w[2] = -2 * C2D * bx, 2 * C2D * by
                w[0] = C2D * (bx * bx - by * by)
        W[:, j] = K[j] * w

    g = n_j - l_eff
    lean = bool(np.all(np.abs(g) < 1e-12))

    phi_h, phi_l = _split_hilo(phi, bf16)
    r2_h, r2_l = _split_hilo(r2A, bf16)

    onehot = np.zeros((NATOMS, NBAS))
    onehot[a_j, np.arange(NBAS)] = 1.0

    W_h = W.astype(bf16)
    W_l = (W - W_h.astype(np.float64)).astype(bf16)

    ah = alpha.astype(bf16)
    al = (alpha - ah.astype(np.float64)).astype(np.float64)

    # t block (rows 0..KT): -(ah+al)(r2h+r2l) to ~2^-16 relative, plus
    # q*ln(r2) levels for the r^g factor when not lean
    rows_t = [r2_h, r2_l, r2_h]                           # 48 rows
    wt_blocks = [onehot * (-ah.astype(np.float64)),
                 onehot * (-ah.astype(np.float64)),
                 onehot * (-al)]
    if not lean:
        lnA = np.log(np.maximum(r2A, 1e-300))
        ln_h, ln_l = _split_hilo(lnA, bf16)
        q = 0.5 * g
        qh = q.astype(bf16)
        ql = (q - qh.astype(np.float64)).astype(np.float64)
        rows_t += [ln_h, ln_l]                            # +32 rows
        wt_blocks += [onehot * qh.astype(np.float64),
                      onehot * qh.astype(np.float64)]
        if not np.allclose(ql, 0):
            # fold the residual-q * ln_h correction into the last level
            wt_blocks[-1] = onehot * (qh.astype(np.float64) + ql)

    rows_p = [phi_h, phi_l, phi_h]                        # 30 rows
    wp_blocks = [W_h, W_h, W_l]

    WT = np.concatenate(wt_blocks).astype(bf16)           # [<=80, NBAS]
    WP = np.concatenate(wp_blocks).astype(bf16)           # [30, NBAS]
    nt, npr = WT.shape[0], WP.shape[0]
    assert nt <= KT and npr <= KP

    rhs = np.zeros((128, NTOT), bf16)
    rhs[0:nt] = np.concatenate(rows_t)
    rhs[KT:KT + npr] = np.concatenate(rows_p)

    # weights: 4 x [128, 128] K=128 blocks: WT-h0 | WT-h1 | WP-h0 | WP-h1.
    # Rows outside each feature block are zero, so one K=128 matmul per
    # (kind, half) selects its rows (full-array matmuls keep the PE at
    # the fast HAM clock; low-K matmuls pin it at 1.2 GHz).
    WB = np.zeros((128, 4 * 128), bf16)
    for h in range(2):
        WB[0:nt, h * 128:(h + 1) * 128] = WT[:, h * 128:(h + 1) * 128]
        WB[KT:KT + npr, (2 + h) * 128:(3 + h) * 128] = \
            WP[:, h * 128:(h + 1) * 128]

    return np.ascontiguousarray(rhs), np.ascontiguousarray(WB)


def _build_nc():
    import concourse.bacc as bacc
    import concourse.mybir as mybir
    import concourse.tile as tile

    f32 = mybir.dt.float32
    f16 = mybir.dt.float16
    bf = mybir.dt.bfloat16

    nc = bacc.Bacc("TRN2", target_bir_lowering=False, debug=False,
                   num_devices=N_CORES)
    rhs_d = nc.dram_tensor("rhs", [128, EPC], bf, kind="ExternalInput")
    wb_d = nc.dram_tensor("wb", [128, 512], bf, kind="ExternalInput")
    out_d = nc.dram_tensor("out", [128, 2 * EPC], f16, kind="ExternalOutput")

    with tile.TileContext(nc) as tc:
        with (
            tc.tile_pool(name="wpool", bufs=1) as wpool,
            tc.tile_pool(name="inpool", bufs=1) as inpool,
            tc.tile_pool(name="upool", bufs=4) as upool,
            tc.tile_pool(name="baspool", bufs=8) as baspool,
            tc.tile_pool(name="ps", bufs=4, space="PSUM") as ps,
        ):
            rt = inpool.tile([128, EPC], bf, tag="rt")
            wb_t = wpool.tile([128, 512], bf, tag="wb")
            warm = wpool.tile([128, 32], bf, tag="warm")
            udum = wpool.tile([128, 32], f32, tag="udum")

            # ACT Exp table preload (~2.7us) overlaps the input DMA wait:
            # memset a tiny tile on gpsimd, exp it on scalar immediately
            nc.gpsimd.memset(warm[:], 0.0)
            nc.scalar.activation(udum[:], warm[:],
                                 mybir.ActivationFunctionType.Exp)

            # startup loads: weights first (tiny), then rhs in rising sizes
            nc.sync.dma_start(wb_t[:], wb_d[:])
            nc.scalar.dma_start(rt[:, 0:EBLK], rhs_d[:, 0:EBLK])
            nc.scalar.dma_start(rt[:, EBLK:2 * EBLK],
                                rhs_d[:, EBLK:2 * EBLK])
            nc.sync.dma_start(rt[:, 2 * EBLK:4 * EBLK],
                              rhs_d[:, 2 * EBLK:4 * EBLK])
            nc.scalar.dma_start(rt[:, 4 * EBLK:8 * EBLK],
                                rhs_d[:, 4 * EBLK:8 * EBLK])
            nc.sync.dma_start(rt[:, 8 * EBLK:], rhs_d[:, 8 * EBLK:])

            for c in range(NCHUNK):
                es = c * EBLK
                tt = ps.tile([128, 2 * EBLK], f32, tag="ps")
                for h in range(2):
                    nc.tensor.matmul(tt[:, h * EBLK:(h + 1) * EBLK],
                                     wb_t[:, h * 128:(h + 1) * 128],
                                     rt[:, es:es + EBLK],
                                     start=True, stop=True)
                pt = ps.tile([128, 2 * EBLK], f32, tag="ps")
                for h in range(2):
                    nc.tensor.matmul(pt[:, h * EBLK:(h + 1) * EBLK],
                                     wb_t[:, (2 + h) * 128:(3 + h) * 128],
                                     rt[:, es:es + EBLK],
                                     start=True, stop=True)
                u = upool.tile([128, 2 * EBLK], f16, tag="u")
                nc.scalar.activation(u[:], tt[:],
                                     mybir.ActivationFunctionType.Exp)
                b = baspool.tile([128, 2 * EBLK], f16, tag="bas")
                nc.vector.tensor_mul(b[:], pt[:], u[:])
                nc.sync.dma_start(out_d[:, 2 * es:2 * es + 2 * EBLK], b[:])

    nc.compile()
    return nc


def kernel(input, atom_coords, bas_exp, bas_coeffs, norm_cst, bas_n,
           bas_l, bas_m, bas_atom_index, index_ctr, _res_hook=None):
    from concourse.bass_utils import run_bass_kernel_spmd

    rhs, WB = _host_build(
        input, atom_coords, bas_exp, bas_coeffs, norm_cst, bas_n,
        bas_l, bas_m, bas_atom_index, index_ctr)

    if "nc" not in _compiled:
        _compiled["nc"] = _build_nc()
    nc = _compiled["nc"]

    in_maps = []
    for i in range(N_CORES):
        es = slice(i * EPC, (i + 1) * EPC)
        in_maps.append({
            "rhs": np.ascontiguousarray(rhs[:, es]),
            "wb": WB,
        })

    res = run_bass_kernel_spmd(nc, in_maps, list(range(N_CORES)))
    if _res_hook is not None:
        _res_hook(res)

    # host-side scatter: index_ctr is sorted, so each orbital's bases are
    # a contiguous run -> segment sums via reduceat
    ictr = np.asarray(index_ctr, np.int64)
    present, first = np.unique(ictr, return_index=True)

    out = np.empty((NBATCH, NELEC, NORB), np.float32)
    for i in range(N_CORES):
        blk = res.results[i]["out"]                  # [128, 2*EPC] f16
        # chunk layout: [128, NCHUNK, 2 halves, EBLK] -> [256, EPC]
        bas = np.ascontiguousarray(
            blk.reshape(128, NCHUNK, 2, EBLK).transpose(2, 0, 1, 3)
        ).reshape(NBAS, EPC).astype(np.float32)
        sums = np.add.reduceat(bas, first, axis=0)   # [npresent, EPC]
        ao = np.zeros((NORB, EPC), np.float32)
        ao[present] = sums
        out[i * BPC:(i + 1) * BPC] = ao.T.reshape(BPC, NELEC, NORB)
    return out
